# revision 1
# baseline (speedup 1.0000x reference)
"""Trainium2 Bass kernel for ContentPopularityJointAttention.

Computes, for each batch row b:
    mp     = concat(m[b], p[b])            # (50, 512)
    hidden = tanh(mp @ Wu)                 # (50, 512)
    s      = hidden @ bvec                 # (50,)
    u[b]   = (sum_n s_n * m[b,n]) / (sum_n s_n)   # (256,)

Sharding: pure data parallel over the batch dim across 8 NeuronCores.

Per-core dataflow (tokens = batch-rows*50 = 25600, processed in 128-token
chunks):
  1. DMA m,p chunk into one [128, 512] fp32 SBUF tile (token-major).
  2. 4 PE transposes -> PSUM [128(d), 512(tok-chunked)] fp32  (feature-major).
  3. fp16 hi/lo split of the transposed data (precision: the sum-normalized
     attention amplifies error ~1/|sum s|, so the hidden matmul needs
     ~fp32-grade products; a 3-term fp16 split reaches 4e-4 rel error
     at 3 cycles/row instead of fp32's 4).
  4. 12 fp16 matmuls: (hiT@Wu_hi + loT@Wu_hi + hiT@Wu_lo), Wu moving,
     mpT chunks stationary -> hidden [128(tok), 512] fp32 PSUM.
  5. ACT tanh -> SBUF fp32.
  6. DVE tensor_tensor_reduce with b replicated across partitions ->
     per-token scores s [128, 1] fp32 (products in fp32).
  7. DVE s * block-diagonal row mask -> lhsT [128, 68] fp16; one fp16
     pooling matmul with rhs = [m | ones] accumulates [sum s*m | sum s]
     into a 64-row group PSUM (rows of 50 tokens may straddle chunks;
     PSUM accumulation handles the overlap).
  8. Per 64-row group: DVE reciprocal + scale -> u rows, DMA out.
"""

import numpy as np
from contextlib import ExitStack

import concourse.bass as bass
import concourse.bacc as bacc
import concourse.tile as tile
from concourse import mybir
from concourse.bass_utils import run_bass_kernel_spmd

N_CORES = 8
B_FULL, N_TOK, MD, PD = 4096, 50, 256, 256
D = MD + PD          # 512 contraction dim
K = 512              # hidden dim
CHUNK = 128          # tokens per chunk (partition dim)
GROUP_ROWS = 64      # batch rows per pooling PSUM accumulation group
GROUP_CHUNKS = GROUP_ROWS * N_TOK // CHUNK   # 25
POOL_P = 68          # pooling PSUM partitions (max local row 63 + span 4)

f32 = mybir.dt.float32
f16 = mybir.dt.float16
bf16 = mybir.dt.bfloat16


def build_program(b_shard: int):
    """Build the single-core Bass program (SPMD: same program, all cores)."""
    tokens = b_shard * N_TOK
    assert tokens % (CHUNK * GROUP_CHUNKS) == 0
    n_groups = b_shard // GROUP_ROWS

    nc = bacc.Bacc("TRN2", target_bir_lowering=False, debug=False,
                   num_devices=N_CORES)

    m_d = nc.dram_tensor("m", [tokens, MD], f32, kind="ExternalInput").ap()
    p_d = nc.dram_tensor("p", [tokens, PD], f32, kind="ExternalInput").ap()
    wu_hi_d = nc.dram_tensor("wu_hi", [128, 4, K], f16, kind="ExternalInput").ap()
    wu_lo_d = nc.dram_tensor("wu_lo", [128, 4, K], f16, kind="ExternalInput").ap()
    brep_d = nc.dram_tensor("brep", [128, K], f32, kind="ExternalInput").ap()
    ident_d = nc.dram_tensor("ident", [128, 128], f32, kind="ExternalInput").ap()
    masks_d = nc.dram_tensor("masks", [128, GROUP_CHUNKS, POOL_P], f16,
                             kind="ExternalInput").ap()
    u_d = nc.dram_tensor("u", [b_shard, MD], f32, kind="ExternalOutput").ap()

    with tile.TileContext(nc) as tc, ExitStack() as ctx:
        singles = ctx.enter_context(tc.tile_pool(name="singles", bufs=1))
        io_pool = ctx.enter_context(tc.tile_pool(name="io", bufs=4))
        work = ctx.enter_context(tc.tile_pool(name="work", bufs=3))
        psum_t = ctx.enter_context(tc.tile_pool(name="psumT", bufs=2, space="PSUM"))
        psum_h = ctx.enter_context(tc.tile_pool(name="psumH", bufs=2, space="PSUM"))
        psum_u = ctx.enter_context(tc.tile_pool(name="psumU", bufs=2, space="PSUM"))

        wu_hi_sb = singles.tile([128, 4, K], f16)
        nc.gpsimd.dma_start(out=wu_hi_sb[:], in_=wu_hi_d)
        wu_lo_sb = singles.tile([128, 4, K], f16)
        nc.gpsimd.dma_start(out=wu_lo_sb[:], in_=wu_lo_d)
        brep_sb = singles.tile([128, K], f32)
        nc.gpsimd.dma_start(out=brep_sb[:], in_=brep_d)
        ident_sb = singles.tile([128, 128], f32)
        nc.gpsimd.dma_start(out=ident_sb[:], in_=ident_d)
        masks_sb = singles.tile([128, GROUP_CHUNKS, POOL_P], f16)
        nc.gpsimd.dma_start(out=masks_sb[:], in_=masks_d)

        for g in range(n_groups):
            pool_ps = psum_u.tile([POOL_P, MD + 1], f32)
            for l in range(GROUP_CHUNKS):
                c = g * GROUP_CHUNKS + l
                t0 = c * CHUNK

                mp32 = io_pool.tile([128, D], f32)
                nc.gpsimd.dma_start(out=mp32[:, 0:MD], in_=m_d[t0:t0 + CHUNK, :])
                nc.gpsimd.dma_start(out=mp32[:, MD:D], in_=p_d[t0:t0 + CHUNK, :])

                # transpose to feature-major
                psT = psum_t.tile([128, D], f32)
                for j in range(4):
                    nc.tensor.transpose(
                        psT[:, j * 128:(j + 1) * 128],
                        mp32[:, j * 128:(j + 1) * 128],
                        ident_sb[:],
                    )

                # fp16 hi/lo split (in transposed domain)
                mpT_hi = work.tile([128, D], f16)
                nc.scalar.copy(out=mpT_hi[:], in_=psT[:])
                mpT_hi32 = work.tile([128, D], f32)
                nc.gpsimd.tensor_copy(out=mpT_hi32[:], in_=mpT_hi[:])
                mpT_lo = work.tile([128, D], f16)
                nc.vector.tensor_sub(mpT_lo[:], psT[:], mpT_hi32[:])

                # hidden = tanh(mp @ Wu), 3-term fp16 split
                hid = psum_h.tile([128, K], f32)
                n_mm = 12
                i_mm = 0
                for lhs in (mpT_hi, mpT_lo):
                    for j in range(4):
                        nc.tensor.matmul(
                            hid[:],
                            lhsT=lhs[:, j * 128:(j + 1) * 128],
                            rhs=wu_hi_sb[:, j, :],
                            start=(i_mm == 0),
                            stop=(i_mm == n_mm - 1),
                        )
                        i_mm += 1
                for j in range(4):
                    nc.tensor.matmul(
                        hid[:],
                        lhsT=mpT_hi[:, j * 128:(j + 1) * 128],
                        rhs=wu_lo_sb[:, j, :],
                        start=(i_mm == 0),
                        stop=(i_mm == n_mm - 1),
                    )
                    i_mm += 1

                tanhH = work.tile([128, K], f32)
                nc.scalar.activation(out=tanhH[:], in_=hid[:],
                                     func=mybir.ActivationFunctionType.Tanh)

                # s[tok] = sum_k tanhH * b   (fp32 products on DVE;
                # tensor_tensor_reduce crashes NRT in this env, use two ops)
                scr = work.tile([128, K], f32)
                s = work.tile([128, 1], f32)
                nc.vector.tensor_mul(scr[:], tanhH[:], brep_sb[:])
                nc.vector.reduce_sum(s[:], scr[:], axis=mybir.AxisListType.X)

                # block-diagonal pooling lhsT and [m | 1] rhs (fp32: the
                # ones-column sum S is cancellation-amplified, fp16 is not
                # enough there)
                blk = work.tile([128, POOL_P], f32)
                nc.vector.tensor_scalar_mul(blk[:], masks_sb[:, l, :], s[:])
                m16 = work.tile([128, MD + 1], f32)
                nc.gpsimd.tensor_copy(out=m16[:, 0:MD], in_=mp32[:, 0:MD])
                nc.vector.memset(m16[:, MD:MD + 1], 1.0)
                nc.tensor.matmul(
                    pool_ps[:],
                    lhsT=blk[:],
                    rhs=m16[:],
                    start=(l == 0),
                    stop=(l == GROUP_CHUNKS - 1),
                )

            rS = work.tile([GROUP_ROWS, 1], f32)
            nc.vector.reciprocal(rS[:], pool_ps[0:GROUP_ROWS, MD:MD + 1])
            u_sb = io_pool.tile([GROUP_ROWS, MD], f32)
            nc.vector.tensor_scalar_mul(u_sb[:], pool_ps[0:GROUP_ROWS, 0:MD], rS[:])
            nc.gpsimd.dma_start(
                out=u_d[g * GROUP_ROWS:(g + 1) * GROUP_ROWS, :], in_=u_sb[:])

    nc.compile()
    return nc


def host_constants(Wu: np.ndarray, b: np.ndarray):
    Wu = np.asarray(Wu, np.float32)
    b = np.asarray(b, np.float32)
    wu_hi16 = Wu.astype(np.float16)
    wu_lo16 = (Wu - wu_hi16.astype(np.float32)).astype(np.float16)
    # [d, k] -> [d%128, d//128, k]
    wu_hi = np.ascontiguousarray(wu_hi16.reshape(4, 128, K).transpose(1, 0, 2))
    wu_lo = np.ascontiguousarray(wu_lo16.reshape(4, 128, K).transpose(1, 0, 2))
    brep = np.ascontiguousarray(np.broadcast_to(b, (128, K)))
    ident = np.eye(128, dtype=np.float32)
    tp = np.arange(128)[:, None, None]
    ll = np.arange(GROUP_CHUNKS)[None, :, None]
    rr = np.arange(POOL_P)[None, None, :]
    masks = (((CHUNK * ll + tp) // N_TOK) == rr).astype(np.float16)
    return {"wu_hi": wu_hi, "wu_lo": wu_lo, "brep": brep, "ident": ident,
            "masks": masks}


_prog_cache: dict = {}


def get_program(b_shard: int):
    if b_shard not in _prog_cache:
        _prog_cache[b_shard] = build_program(b_shard)
    return _prog_cache[b_shard]


def kernel(m: np.ndarray, p: np.ndarray, Wu: np.ndarray, b: np.ndarray
           ) -> np.ndarray:
    m = np.ascontiguousarray(np.asarray(m, np.float32))
    p = np.ascontiguousarray(np.asarray(p, np.float32))
    B = m.shape[0]
    assert B % N_CORES == 0
    b_shard = B // N_CORES

    nc = get_program(b_shard)
    consts = host_constants(Wu, b)

    mf = m.reshape(B * N_TOK, MD)
    pf = p.reshape(B * N_TOK, PD)
    tok_sh = b_shard * N_TOK
    in_maps = []
    for c in range(N_CORES):
        in_maps.append({
            "m": mf[c * tok_sh:(c + 1) * tok_sh],
            "p": pf[c * tok_sh:(c + 1) * tok_sh],
            **consts,
        })
    res = run_bass_kernel_spmd(nc, in_maps, list(range(N_CORES)))
    u = np.concatenate([res.results[c]["u"] for c in range(N_CORES)], axis=0)
    return u.astype(np.float32)



# revision 2
# speedup vs baseline: 1.2536x; 1.2536x over previous
"""Trainium2 Bass kernel for ContentPopularityJointAttention.

Computes, for each batch row b:
    mp     = concat(m[b], p[b])            # (50, 512)
    hidden = tanh(mp @ Wu)                 # (50, 512)
    s      = hidden @ bvec                 # (50,)
    u[b]   = (sum_n s_n * m[b,n]) / (sum_n s_n)   # (256,)

Sharding: pure data parallel over the batch dim across 8 NeuronCores.

v2 changes vs the first working kernel (703us):
  - mp is transposed and fp16 hi/lo-split on the HOST; the kernel loads
    mpT_hi/mpT_lo feature-major straight from DRAM.  This removes the 4
    PE transposes, the ACT hi-copy, the Pool hi32-copy and the DVE lo-sub
    that used to run every chunk.
  - the pooling matmul runs in fp16 (1 cycle/row instead of 4) against a
    host-prepared token-major [m|...] fp16 tensor; the cancellation-
    sensitive ones-column sum S is accumulated by a separate tiny fp32
    matmul (free size 1, 4 cycles), keeping the denominator fp32-grade.
  - inputs stream in one group (64 rows = 25 chunks) at a time via three
    large HWDGE DMAs issued from the otherwise-idle SP engine, instead of
    two SWDGE DMAs per chunk on Pool (~2us/chunk of Pool time).

Per-core dataflow (tokens = 512 batch-rows * 50 = 25600, in groups of
3200 tokens = 25 chunks of 128):
  1. SP DMA group tiles: mpT_hi/lo [128, 4, 3200] fp16 (feature-major),
     mt [128, 25, 257] fp16 (token-major [m | ones]).
  2. Per chunk: 12 fp16 matmuls (hi@Wu_hi + lo@Wu_hi + hi@Wu_lo), Wu
     moving, mpT chunk slices stationary -> hidden [128 tok, 512] PSUM.
  3. ACT tanh -> SBUF fp32.
  4. DVE mul by b (replicated) + reduce -> s [128, 1] fp32.
  5. DVE s * block-diagonal row mask -> blk16 [128, 68] fp16 and
     blk32 [128, 68] fp32.
  6. fp16 pooling matmul (blk16 x mt[:, :256]) accumulates sum s*m into
     PSUM [68, 256]; fp32 matmul (blk32 x ones) accumulates S into
     PSUM [68, 1] (rows of 50 tokens may straddle chunks; PSUM
     accumulation handles the overlap).
  7. Per 64-row group: DVE reciprocal + scale -> u rows, Pool DMA out.
"""

import numpy as np
from contextlib import ExitStack

import concourse.bass as bass
import concourse.bacc as bacc
import concourse.tile as tile
from concourse import mybir
from concourse.bass_utils import run_bass_kernel_spmd

N_CORES = 8
B_FULL, N_TOK, MD, PD = 4096, 50, 256, 256
D = MD + PD          # 512 contraction dim
K = 512              # hidden dim
CHUNK = 128          # tokens per chunk (partition dim)
GROUP_ROWS = 64      # batch rows per pooling PSUM accumulation group
GROUP_CHUNKS = GROUP_ROWS * N_TOK // CHUNK   # 25
GROUP_TOK = GROUP_CHUNKS * CHUNK             # 3200
POOL_P = 68          # pooling PSUM partitions (max local row 63 + span 4)

f32 = mybir.dt.float32
f16 = mybir.dt.float16


def build_program(b_shard: int):
    """Build the single-core Bass program (SPMD: same program, all cores)."""
    tokens = b_shard * N_TOK
    assert tokens % GROUP_TOK == 0
    n_groups = tokens // GROUP_TOK

    nc = bacc.Bacc("TRN2", target_bir_lowering=False, debug=False,
                   num_devices=N_CORES)

    mpthi_d = nc.dram_tensor("mpThi", [128, 4, tokens], f16,
                             kind="ExternalInput").ap()
    mptlo_d = nc.dram_tensor("mpTlo", [128, 4, tokens], f16,
                             kind="ExternalInput").ap()
    mt_d = nc.dram_tensor("mt", [tokens, MD + 1], f16,
                          kind="ExternalInput").ap()
    wu_hi_d = nc.dram_tensor("wu_hi", [128, 4, K], f16, kind="ExternalInput").ap()
    wu_lo_d = nc.dram_tensor("wu_lo", [128, 4, K], f16, kind="ExternalInput").ap()
    brep_d = nc.dram_tensor("brep", [128, K], f32, kind="ExternalInput").ap()
    ones_d = nc.dram_tensor("ones32", [128, 1], f32, kind="ExternalInput").ap()
    masks_d = nc.dram_tensor("masks", [128, GROUP_CHUNKS, POOL_P], f16,
                             kind="ExternalInput").ap()
    u_d = nc.dram_tensor("u", [b_shard, MD], f32, kind="ExternalOutput").ap()

    with tile.TileContext(nc) as tc, ExitStack() as ctx:
        singles = ctx.enter_context(tc.tile_pool(name="singles", bufs=1))
        in_pool = ctx.enter_context(tc.tile_pool(name="inp", bufs=2))
        io_pool = ctx.enter_context(tc.tile_pool(name="io", bufs=2))
        work = ctx.enter_context(tc.tile_pool(name="work", bufs=3))
        psum_h = ctx.enter_context(tc.tile_pool(name="psumH", bufs=2, space="PSUM"))
        psum_u = ctx.enter_context(tc.tile_pool(name="psumU", bufs=2, space="PSUM"))

        wu_hi_sb = singles.tile([128, 4, K], f16)
        nc.gpsimd.dma_start(out=wu_hi_sb[:], in_=wu_hi_d)
        wu_lo_sb = singles.tile([128, 4, K], f16)
        nc.gpsimd.dma_start(out=wu_lo_sb[:], in_=wu_lo_d)
        brep_sb = singles.tile([128, K], f32)
        nc.gpsimd.dma_start(out=brep_sb[:], in_=brep_d)
        ones_sb = singles.tile([128, 1], f32)
        nc.gpsimd.dma_start(out=ones_sb[:], in_=ones_d)
        masks_sb = singles.tile([128, GROUP_CHUNKS, POOL_P], f16)
        nc.gpsimd.dma_start(out=masks_sb[:], in_=masks_d)

        for g in range(n_groups):
            t0 = g * GROUP_TOK
            hi_sb = in_pool.tile([128, 4, GROUP_TOK], f16)
            nc.sync.dma_start(out=hi_sb[:], in_=mpthi_d[:, :, t0:t0 + GROUP_TOK])
            lo_sb = in_pool.tile([128, 4, GROUP_TOK], f16)
            nc.sync.dma_start(out=lo_sb[:], in_=mptlo_d[:, :, t0:t0 + GROUP_TOK])
            mt_sb = in_pool.tile([128, GROUP_CHUNKS, MD + 1], f16)
            nc.sync.dma_start(
                out=mt_sb[:],
                in_=mt_d[t0:t0 + GROUP_TOK, :].rearrange(
                    "(c p) f -> p c f", p=CHUNK),
            )

            pool_m = psum_u.tile([POOL_P, MD], f32)
            pool_s = psum_u.tile([POOL_P, 1], f32)
            for l in range(GROUP_CHUNKS):
                c0 = l * CHUNK

                # hidden = tanh(mp @ Wu), 3-term fp16 split
                hid = psum_h.tile([128, K], f32)
                n_mm = 12
                i_mm = 0
                for lhs in (hi_sb, lo_sb):
                    for j in range(4):
                        nc.tensor.matmul(
                            hid[:],
                            lhsT=lhs[:, j, c0:c0 + CHUNK],
                            rhs=wu_hi_sb[:, j, :],
                            start=(i_mm == 0),
                            stop=(i_mm == n_mm - 1),
                        )
                        i_mm += 1
                for j in range(4):
                    nc.tensor.matmul(
                        hid[:],
                        lhsT=hi_sb[:, j, c0:c0 + CHUNK],
                        rhs=wu_lo_sb[:, j, :],
                        start=(i_mm == 0),
                        stop=(i_mm == n_mm - 1),
                    )
                    i_mm += 1

                tanhH = work.tile([128, K], f32)
                nc.scalar.activation(out=tanhH[:], in_=hid[:],
                                     func=mybir.ActivationFunctionType.Tanh)

                # s[tok] = sum_k tanhH * b   (fp32 products on DVE)
                scr = work.tile([128, K], f32)
                s = work.tile([128, 1], f32)
                nc.vector.tensor_mul(scr[:], tanhH[:], brep_sb[:])
                nc.vector.reduce_sum(s[:], scr[:], axis=mybir.AxisListType.X)

                # block-diagonal pooling lhsT; fp16 for the m columns,
                # fp32 for the cancellation-amplified ones-column sum S
                blk16 = work.tile([128, POOL_P], f16)
                nc.vector.tensor_scalar_mul(blk16[:], masks_sb[:, l, :], s[:])
                blk32 = work.tile([128, POOL_P], f32)
                nc.vector.tensor_scalar_mul(blk32[:], masks_sb[:, l, :], s[:])
                nc.tensor.matmul(
                    pool_m[:],
                    lhsT=blk16[:],
                    rhs=mt_sb[:, l, 0:MD],
                    start=(l == 0),
                    stop=(l == GROUP_CHUNKS - 1),
                )
                nc.tensor.matmul(
                    pool_s[:],
                    lhsT=blk32[:],
                    rhs=ones_sb[:],
                    start=(l == 0),
                    stop=(l == GROUP_CHUNKS - 1),
                )

            rS = work.tile([GROUP_ROWS, 1], f32)
            nc.vector.reciprocal(rS[:], pool_s[0:GROUP_ROWS, :])
            u_sb = io_pool.tile([GROUP_ROWS, MD], f32)
            nc.vector.tensor_scalar_mul(u_sb[:], pool_m[0:GROUP_ROWS, :], rS[:])
            nc.gpsimd.dma_start(
                out=u_d[g * GROUP_ROWS:(g + 1) * GROUP_ROWS, :], in_=u_sb[:])

    nc.compile()
    return nc


def host_constants(Wu: np.ndarray, b: np.ndarray):
    Wu = np.asarray(Wu, np.float32)
    b = np.asarray(b, np.float32)
    wu_hi16 = Wu.astype(np.float16)
    wu_lo16 = (Wu - wu_hi16.astype(np.float32)).astype(np.float16)
    # [d, k] -> [d%128, d//128, k]
    wu_hi = np.ascontiguousarray(wu_hi16.reshape(4, 128, K).transpose(1, 0, 2))
    wu_lo = np.ascontiguousarray(wu_lo16.reshape(4, 128, K).transpose(1, 0, 2))
    brep = np.ascontiguousarray(np.broadcast_to(b, (128, K)))
    ones32 = np.ones((128, 1), np.float32)
    tp = np.arange(128)[:, None, None]
    ll = np.arange(GROUP_CHUNKS)[None, :, None]
    rr = np.arange(POOL_P)[None, None, :]
    masks = (((CHUNK * ll + tp) // N_TOK) == rr).astype(np.float16)
    return {"wu_hi": wu_hi, "wu_lo": wu_lo, "brep": brep, "ones32": ones32,
            "masks": masks}


def host_shard_inputs(mf: np.ndarray, pf: np.ndarray):
    """Per-shard token tensors.

    mf, pf: [tokens, 256] fp32 (token-major).  Returns the feature-major
    fp16 hi/lo split of concat(m, p) and the token-major [m | 1] fp16.
    """
    tokens = mf.shape[0]
    mp = np.concatenate([mf, pf], axis=1)          # [tokens, 512]
    hi = mp.astype(np.float16)
    lo = (mp - hi.astype(np.float32)).astype(np.float16)
    # [tokens, 512] -> [128, 4, tokens]
    mpthi = np.ascontiguousarray(hi.T.reshape(4, 128, tokens).transpose(1, 0, 2))
    mptlo = np.ascontiguousarray(lo.T.reshape(4, 128, tokens).transpose(1, 0, 2))
    mt = np.empty((tokens, MD + 1), np.float16)
    mt[:, 0:MD] = mf.astype(np.float16)
    mt[:, MD] = 1.0
    return {"mpThi": mpthi, "mpTlo": mptlo, "mt": mt}


_prog_cache: dict = {}


def get_program(b_shard: int):
    if b_shard not in _prog_cache:
        _prog_cache[b_shard] = build_program(b_shard)
    return _prog_cache[b_shard]


def kernel(m: np.ndarray, p: np.ndarray, Wu: np.ndarray, b: np.ndarray
           ) -> np.ndarray:
    m = np.ascontiguousarray(np.asarray(m, np.float32))
    p = np.ascontiguousarray(np.asarray(p, np.float32))
    B = m.shape[0]
    assert B % N_CORES == 0
    b_shard = B // N_CORES

    nc = get_program(b_shard)
    consts = host_constants(Wu, b)

    mf = m.reshape(B * N_TOK, MD)
    pf = p.reshape(B * N_TOK, PD)
    tok_sh = b_shard * N_TOK
    in_maps = []
    for c in range(N_CORES):
        in_maps.append({
            **host_shard_inputs(mf[c * tok_sh:(c + 1) * tok_sh],
                                pf[c * tok_sh:(c + 1) * tok_sh]),
            **consts,
        })
    res = run_bass_kernel_spmd(nc, in_maps, list(range(N_CORES)))
    u = np.concatenate([res.results[c]["u"] for c in range(N_CORES)], axis=0)
    return u.astype(np.float32)


# revision 3
# speedup vs baseline: 1.2906x; 1.0295x over previous
"""Trainium2 Bass kernel for ContentPopularityJointAttention.

Computes, for each batch row b:
    mp     = concat(m[b], p[b])            # (50, 512)
    hidden = tanh(mp @ Wu)                 # (50, 512)
    s      = hidden @ bvec                 # (50,)
    u[b]   = (sum_n s_n * m[b,n]) / (sum_n s_n)   # (256,)

Sharding: pure data parallel over the batch dim across 8 NeuronCores.

v2 changes vs the first working kernel (703us):
  - mp is transposed and fp16 hi/lo-split on the HOST; the kernel loads
    mpT_hi/mpT_lo feature-major straight from DRAM.  This removes the 4
    PE transposes, the ACT hi-copy, the Pool hi32-copy and the DVE lo-sub
    that used to run every chunk.
  - the pooling matmul runs in fp16 (1 cycle/row instead of 4) against a
    host-prepared token-major [m|...] fp16 tensor; the cancellation-
    sensitive ones-column sum S is accumulated by a separate tiny fp32
    matmul (free size 1, 4 cycles), keeping the denominator fp32-grade.
  - inputs stream in one group (64 rows = 25 chunks) at a time via three
    large HWDGE DMAs issued from the otherwise-idle SP engine, instead of
    two SWDGE DMAs per chunk on Pool (~2us/chunk of Pool time).

Per-core dataflow (tokens = 512 batch-rows * 50 = 25600, in groups of
3200 tokens = 25 chunks of 128):
  1. SP DMA group tiles: mpT_hi/lo [128, 4, 3200] fp16 (feature-major),
     mt [128, 25, 257] fp16 (token-major [m | ones]).
  2. Per chunk: 12 fp16 matmuls (hi@Wu_hi + lo@Wu_hi + hi@Wu_lo), Wu
     moving, mpT chunk slices stationary -> hidden [128 tok, 512] PSUM.
  3. ACT tanh -> SBUF fp32.
  4. DVE mul by b (replicated) + reduce -> s [128, 1] fp32.
  5. DVE s * block-diagonal row mask -> blk16 [128, 68] fp16 and
     blk32 [128, 68] fp32.
  6. fp16 pooling matmul (blk16 x mt[:, :256]) accumulates sum s*m into
     PSUM [68, 256]; fp32 matmul (blk32 x ones) accumulates S into
     PSUM [68, 1] (rows of 50 tokens may straddle chunks; PSUM
     accumulation handles the overlap).
  7. Per 64-row group: DVE reciprocal + scale -> u rows, Pool DMA out.
"""

import numpy as np
from contextlib import ExitStack

import concourse.bass as bass
import concourse.bacc as bacc
import concourse.tile as tile
from concourse import mybir
from concourse.bass_utils import run_bass_kernel_spmd

N_CORES = 8
B_FULL, N_TOK, MD, PD = 4096, 50, 256, 256
D = MD + PD          # 512 contraction dim
K = 512              # hidden dim
CHUNK = 128          # tokens per chunk (partition dim)
GROUP_ROWS = 64      # batch rows per pooling PSUM accumulation group
GROUP_CHUNKS = GROUP_ROWS * N_TOK // CHUNK   # 25
GROUP_TOK = GROUP_CHUNKS * CHUNK             # 3200
POOL_P = 68          # pooling PSUM partitions (max local row 63 + span 4)

f32 = mybir.dt.float32
f16 = mybir.dt.float16


def build_program(b_shard: int):
    """Build the single-core Bass program (SPMD: same program, all cores)."""
    tokens = b_shard * N_TOK
    assert tokens % GROUP_TOK == 0
    n_groups = tokens // GROUP_TOK

    nc = bacc.Bacc("TRN2", target_bir_lowering=False, debug=False,
                   num_devices=N_CORES)

    mpthi_d = nc.dram_tensor("mpThi", [128, 4, tokens], f16,
                             kind="ExternalInput").ap()
    mptlo_d = nc.dram_tensor("mpTlo", [128, 4, tokens], f16,
                             kind="ExternalInput").ap()
    mt_d = nc.dram_tensor("mt", [tokens, MD + 1], f16,
                          kind="ExternalInput").ap()
    wu_hi_d = nc.dram_tensor("wu_hi", [128, 4, K], f16, kind="ExternalInput").ap()
    wu_lo_d = nc.dram_tensor("wu_lo", [128, 4, K], f16, kind="ExternalInput").ap()
    brep_d = nc.dram_tensor("brep", [128, K], f32, kind="ExternalInput").ap()
    ones_d = nc.dram_tensor("ones32", [128, 1], f32, kind="ExternalInput").ap()
    masks_d = nc.dram_tensor("masks", [128, GROUP_CHUNKS, POOL_P], f16,
                             kind="ExternalInput").ap()
    u_d = nc.dram_tensor("u", [b_shard, MD], f32, kind="ExternalOutput").ap()

    with tile.TileContext(nc) as tc, ExitStack() as ctx:
        singles = ctx.enter_context(tc.tile_pool(name="singles", bufs=1))
        in_pool = ctx.enter_context(tc.tile_pool(name="inp", bufs=2))
        io_pool = ctx.enter_context(tc.tile_pool(name="io", bufs=2))
        work = ctx.enter_context(tc.tile_pool(name="work", bufs=3))
        psum_h = ctx.enter_context(tc.tile_pool(name="psumH", bufs=2, space="PSUM"))
        psum_u = ctx.enter_context(tc.tile_pool(name="psumU", bufs=2, space="PSUM"))

        wu_hi_sb = singles.tile([128, 4, K], f16)
        nc.gpsimd.dma_start(out=wu_hi_sb[:], in_=wu_hi_d)
        wu_lo_sb = singles.tile([128, 4, K], f16)
        nc.gpsimd.dma_start(out=wu_lo_sb[:], in_=wu_lo_d)
        brep_sb = singles.tile([128, K], f32)
        nc.gpsimd.dma_start(out=brep_sb[:], in_=brep_d)
        ones_sb = singles.tile([128, 1], f32)
        nc.gpsimd.dma_start(out=ones_sb[:], in_=ones_d)
        masks_sb = singles.tile([128, GROUP_CHUNKS, POOL_P], f16)
        nc.gpsimd.dma_start(out=masks_sb[:], in_=masks_d)

        for g in range(n_groups):
            t0 = g * GROUP_TOK
            hi_sb = in_pool.tile([128, 4, GROUP_TOK], f16)
            lo_sb = in_pool.tile([128, 4, GROUP_TOK], f16)
            mt_sb = in_pool.tile([128, GROUP_CHUNKS, MD + 1], f16)
            # Round-robin sliced loads so chunk 0's compute starts after
            # ~1/5 of the group traffic instead of all of it.
            n_sl = 5
            ch_sl = GROUP_CHUNKS // n_sl
            tk_sl = ch_sl * CHUNK
            for q in range(n_sl):
                q0 = t0 + q * tk_sl
                nc.sync.dma_start(
                    out=hi_sb[:, :, q * tk_sl:(q + 1) * tk_sl],
                    in_=mpthi_d[:, :, q0:q0 + tk_sl])
                nc.sync.dma_start(
                    out=lo_sb[:, :, q * tk_sl:(q + 1) * tk_sl],
                    in_=mptlo_d[:, :, q0:q0 + tk_sl])
                nc.sync.dma_start(
                    out=mt_sb[:, q * ch_sl:(q + 1) * ch_sl, :],
                    in_=mt_d[q0:q0 + tk_sl, :].rearrange(
                        "(c p) f -> p c f", p=CHUNK),
                )

            pool_m = psum_u.tile([POOL_P, MD], f32)
            pool_s = psum_u.tile([POOL_P, 1], f32)
            for l in range(GROUP_CHUNKS):
                c0 = l * CHUNK

                # hidden = tanh(mp @ Wu), 3-term fp16 split
                hid = psum_h.tile([128, K], f32)
                n_mm = 12
                i_mm = 0
                for lhs in (hi_sb, lo_sb):
                    for j in range(4):
                        nc.tensor.matmul(
                            hid[:],
                            lhsT=lhs[:, j, c0:c0 + CHUNK],
                            rhs=wu_hi_sb[:, j, :],
                            start=(i_mm == 0),
                            stop=(i_mm == n_mm - 1),
                        )
                        i_mm += 1
                for j in range(4):
                    nc.tensor.matmul(
                        hid[:],
                        lhsT=hi_sb[:, j, c0:c0 + CHUNK],
                        rhs=wu_lo_sb[:, j, :],
                        start=(i_mm == 0),
                        stop=(i_mm == n_mm - 1),
                    )
                    i_mm += 1

                tanhH = work.tile([128, K], f32)
                nc.scalar.activation(out=tanhH[:], in_=hid[:],
                                     func=mybir.ActivationFunctionType.Tanh)

                # s[tok] = sum_k tanhH * b   (fp32 products on DVE)
                scr = work.tile([128, K], f32)
                s = work.tile([128, 1], f32)
                nc.vector.tensor_mul(scr[:], tanhH[:], brep_sb[:])
                nc.vector.reduce_sum(s[:], scr[:], axis=mybir.AxisListType.X)

                # block-diagonal pooling lhsT; fp16 for the m columns,
                # fp32 for the cancellation-amplified ones-column sum S
                blk16 = work.tile([128, POOL_P], f16)
                nc.vector.tensor_scalar_mul(blk16[:], masks_sb[:, l, :], s[:])
                blk32 = work.tile([128, POOL_P], f32)
                nc.vector.tensor_scalar_mul(blk32[:], masks_sb[:, l, :], s[:])
                nc.tensor.matmul(
                    pool_m[:],
                    lhsT=blk16[:],
                    rhs=mt_sb[:, l, 0:MD],
                    start=(l == 0),
                    stop=(l == GROUP_CHUNKS - 1),
                )
                nc.tensor.matmul(
                    pool_s[:],
                    lhsT=blk32[:],
                    rhs=ones_sb[:],
                    start=(l == 0),
                    stop=(l == GROUP_CHUNKS - 1),
                )

            rS = work.tile([GROUP_ROWS, 1], f32)
            nc.vector.reciprocal(rS[:], pool_s[0:GROUP_ROWS, :])
            u_sb = io_pool.tile([GROUP_ROWS, MD], f32)
            nc.vector.tensor_scalar_mul(u_sb[:], pool_m[0:GROUP_ROWS, :], rS[:])
            nc.gpsimd.dma_start(
                out=u_d[g * GROUP_ROWS:(g + 1) * GROUP_ROWS, :], in_=u_sb[:])

    nc.compile()
    return nc


def host_constants(Wu: np.ndarray, b: np.ndarray):
    Wu = np.asarray(Wu, np.float32)
    b = np.asarray(b, np.float32)
    wu_hi16 = Wu.astype(np.float16)
    wu_lo16 = (Wu - wu_hi16.astype(np.float32)).astype(np.float16)
    # [d, k] -> [d%128, d//128, k]
    wu_hi = np.ascontiguousarray(wu_hi16.reshape(4, 128, K).transpose(1, 0, 2))
    wu_lo = np.ascontiguousarray(wu_lo16.reshape(4, 128, K).transpose(1, 0, 2))
    brep = np.ascontiguousarray(np.broadcast_to(b, (128, K)))
    ones32 = np.ones((128, 1), np.float32)
    tp = np.arange(128)[:, None, None]
    ll = np.arange(GROUP_CHUNKS)[None, :, None]
    rr = np.arange(POOL_P)[None, None, :]
    masks = (((CHUNK * ll + tp) // N_TOK) == rr).astype(np.float16)
    return {"wu_hi": wu_hi, "wu_lo": wu_lo, "brep": brep, "ones32": ones32,
            "masks": masks}


def host_shard_inputs(mf: np.ndarray, pf: np.ndarray):
    """Per-shard token tensors.

    mf, pf: [tokens, 256] fp32 (token-major).  Returns the feature-major
    fp16 hi/lo split of concat(m, p) and the token-major [m | 1] fp16.
    """
    tokens = mf.shape[0]
    mp = np.concatenate([mf, pf], axis=1)          # [tokens, 512]
    hi = mp.astype(np.float16)
    lo = (mp - hi.astype(np.float32)).astype(np.float16)
    # [tokens, 512] -> [128, 4, tokens]
    mpthi = np.ascontiguousarray(hi.T.reshape(4, 128, tokens).transpose(1, 0, 2))
    mptlo = np.ascontiguousarray(lo.T.reshape(4, 128, tokens).transpose(1, 0, 2))
    mt = np.empty((tokens, MD + 1), np.float16)
    mt[:, 0:MD] = mf.astype(np.float16)
    mt[:, MD] = 1.0
    return {"mpThi": mpthi, "mpTlo": mptlo, "mt": mt}


_prog_cache: dict = {}


def get_program(b_shard: int):
    if b_shard not in _prog_cache:
        _prog_cache[b_shard] = build_program(b_shard)
    return _prog_cache[b_shard]


def kernel(m: np.ndarray, p: np.ndarray, Wu: np.ndarray, b: np.ndarray
           ) -> np.ndarray:
    m = np.ascontiguousarray(np.asarray(m, np.float32))
    p = np.ascontiguousarray(np.asarray(p, np.float32))
    B = m.shape[0]
    assert B % N_CORES == 0
    b_shard = B // N_CORES

    nc = get_program(b_shard)
    consts = host_constants(Wu, b)

    mf = m.reshape(B * N_TOK, MD)
    pf = p.reshape(B * N_TOK, PD)
    tok_sh = b_shard * N_TOK
    in_maps = []
    for c in range(N_CORES):
        in_maps.append({
            **host_shard_inputs(mf[c * tok_sh:(c + 1) * tok_sh],
                                pf[c * tok_sh:(c + 1) * tok_sh]),
            **consts,
        })
    res = run_bass_kernel_spmd(nc, in_maps, list(range(N_CORES)))
    u = np.concatenate([res.results[c]["u"] for c in range(N_CORES)], axis=0)
    return u.astype(np.float32)
